# revision 6
# baseline (speedup 1.0000x reference)
"""Trainium2 Bass kernel for the DeepBSDE loss (nn_BaseDeepBSDE).

Data-parallel over 8 NeuronCores: each core simulates 2048 Monte-Carlo
paths through the 100-step SDE loop and produces a partial loss sum;
the host sums the 8 partial scalars.

v2 design (vs v1 baseline):
  - Two software-pipelined path groups (A: chunks 0-7, B: chunks 8-15,
    1024 paths each). Group B's matmuls overlap group A's epilogue so
    the PE stays dense (ramps to full 2.4 GHz pstate instead of 1.2).
  - L1 as K=1 rank-1 matmuls straight from y rows (no block-diag L1b).
  - dd = dW - dZ precomputed once per quarter (residual = z . dd),
    dropping the v-branch of the epilogue.
  - relu work split across Scalar/Vector/GpSimd engines.
  - bf16 PE transpose for the y-update increment.
"""

import os
import sys

sys.path.insert(0, "/opt/trn_rl_repo")

import numpy as np

B = 16384
NSTEPS = 100
DIMW = 3
DT = 0.01
SQRT_DT = DT**0.5
SIGMA0 = 0.5
NCORES = 8
BC = B // NCORES  # 2048 paths per core
NCH = BC // 128  # 16 chunks of 128 paths
NG = 2  # path groups per core
GCH = NCH // NG  # 8 chunks per group
NQ = 4  # noise quarters

LAST_EXEC_NS = None
LAST_RESULTS = None

_CACHE = {}


def _build(nsteps, debug=False):
    import concourse.tile as tile
    from concourse import bacc, mybir

    f32 = mybir.dt.float32
    bf16 = mybir.dt.bfloat16
    AF = mybir.ActivationFunctionType
    ALU = mybir.AluOpType
    AX = mybir.AxisListType

    nc = bacc.Bacc("TRN2", target_bir_lowering=False, debug=False, num_devices=NCORES)

    QSTEPS = (nsteps + NQ - 1) // NQ
    dWf_d = [
        nc.dram_tensor(f"dWf{q}", [128, QSTEPS * 48], f32, kind="ExternalInput").ap()
        for q in range(NQ)
    ]
    dZf_d = [
        nc.dram_tensor(f"dZf{q}", [128, QSTEPS * 48], f32, kind="ExternalInput").ap()
        for q in range(NQ)
    ]
    L1bg_d = nc.dram_tensor("L1bg", [GCH, GCH * 128], f32, kind="ExternalInput").ap()
    W1c_d = nc.dram_tensor("W1c", [2, 128], f32, kind="ExternalInput").ap()
    W2bd_d = nc.dram_tensor("W2bd", [128, 128], f32, kind="ExternalInput").ap()
    W3c_d = nc.dram_tensor("W3c", [128, 4], f32, kind="ExternalInput").ap()
    b1c_d = nc.dram_tensor("b1c", [128, 1], f32, kind="ExternalInput").ap()
    b2c_d = nc.dram_tensor("b2c", [128, 1], f32, kind="ExternalInput").ap()
    b3c_d = nc.dram_tensor("b3c", [1, 4], f32, kind="ExternalInput").ap()
    tvals_d = nc.dram_tensor("tvals", [1, nsteps], f32, kind="ExternalInput").ap()
    ones_col_d = nc.dram_tensor("ones_col", [128, 1], f32, kind="ExternalInput").ap()
    I128_d = nc.dram_tensor("I128", [128, 128], f32, kind="ExternalInput").ap()
    y_init_d = nc.dram_tensor("y_init", [16, 128], f32, kind="ExternalInput").ap()
    Y_init_d = nc.dram_tensor("Y_init", [128, 16], f32, kind="ExternalInput").ap()

    loss_out = nc.dram_tensor("loss_out", [1, 1], f32, kind="ExternalOutput").ap()
    if debug:
        y_out = nc.dram_tensor("y_out", [16, 128], f32, kind="ExternalOutput").ap()
        Y_out = nc.dram_tensor("Y_out", [128, 16], f32, kind="ExternalOutput").ap()
        zq_out = nc.dram_tensor("zq_out", [128, 64], f32, kind="ExternalOutput").ap()

    SC_F = float((0.5 / DT) ** 0.5)  # (SC_F * qDT)^2 = 0.5*dt*q^2

    with tile.TileContext(nc) as tc:
        from contextlib import ExitStack

        with ExitStack() as ctx:
            cpool = ctx.enter_context(tc.tile_pool(name="const", bufs=1))
            h1pool = ctx.enter_context(tc.tile_pool(name="h1sb", bufs=3))
            h2pool = ctx.enter_context(tc.tile_pool(name="h2sb", bufs=3))
            epool = ctx.enter_context(tc.tile_pool(name="epil", bufs=3))
            pmm = ctx.enter_context(tc.tile_pool(name="pmm", bufs=2, space="PSUM"))
            pzq = ctx.enter_context(tc.tile_pool(name="pzq", bufs=1, space="PSUM"))
            ptr = ctx.enter_context(tc.tile_pool(name="ptr", bufs=2, space="PSUM"))
            ploss = ctx.enter_context(tc.tile_pool(name="ploss", bufs=1, space="PSUM"))

            # ------------- persistent SBUF tiles -------------
            dWs = [cpool.tile([128, QSTEPS * 48], f32, tag=f"dw{q}", name=f"dws{q}") for q in range(NQ)]
            dds = [cpool.tile([128, QSTEPS * 48], f32, tag=f"dz{q}", name=f"dds{q}") for q in range(NQ)]
            swp = cpool.tile([128, nsteps * 16], f32, tag="swp")
            W2bd_bf = cpool.tile([128, 128], bf16, tag="w2bd")
            L1bg_bf = cpool.tile([GCH, GCH * 128], bf16, tag="l1bg")
            W3_bf = cpool.tile([128, 4], bf16, tag="w3")
            W3_f = cpool.tile([128, 4], f32, tag="w3f")
            b1tab = cpool.tile([128, nsteps], f32, tag="b1tab")
            b1c_sb = cpool.tile([128, 1], f32, tag="b1c")
            b2c_sb = cpool.tile([128, 1], f32, tag="b2c")
            b3s = cpool.tile([1, 4], f32, tag="b3s")
            b3f = cpool.tile([1, 4], f32, tag="b3f")
            b3rep = cpool.tile([1, 32], bf16, tag="b3rep")
            ones_bf = cpool.tile([1, 128], bf16, tag="ones_bf")
            ones_col = cpool.tile([128, 1], f32, tag="ones_col")
            I128 = cpool.tile([128, 128], f32, tag="i128")
            I128bf = cpool.tile([128, 128], bf16, tag="i128bf")
            W1c_sb = cpool.tile([2, 128], f32, tag="w1c")
            tvals = cpool.tile([1, nsteps], f32, tag="tvals")
            yg = [cpool.tile([GCH, 128], f32, tag=f"y{g}", name=f"yg{g}") for g in range(NG)]
            yg_bf = [cpool.tile([GCH, 128], bf16, tag=f"ybf{g}", name=f"ygbf{g}") for g in range(NG)]
            Yacc = cpool.tile([128, 16], f32, tag="Yacc")
            ysq = [cpool.tile([GCH, 128], f32, tag=f"ysq{g}", name=f"ysq{g}") for g in range(NG)]
            ee = [cpool.tile([128, GCH], f32, tag=f"ee{g}", name=f"ee{g}") for g in range(NG)]
            loss_sb = cpool.tile([1, 16], f32, tag="loss_sb")
            loss1 = cpool.tile([1, 1], f32, tag="loss1")

            loss_ps = ploss.tile([1, 16], f32, tag="loss")

            # ------------- init: DMAs -------------
            for q in range(NQ):
                nc.sync.dma_start(dWs[q][:], dWf_d[q][:])
                nc.sync.dma_start(dds[q][:], dZf_d[q][:])
            nc.gpsimd.dma_start(W2bd_bf[:], W2bd_d[:])
            nc.gpsimd.dma_start(L1bg_bf[:], L1bg_d[:])
            nc.sync.dma_start(W3_f[:], W3c_d[:])
            nc.sync.dma_start(b1c_sb[:], b1c_d[:])
            nc.sync.dma_start(b2c_sb[:], b2c_d[:])
            nc.sync.dma_start(b3f[:], b3c_d[:])
            nc.sync.dma_start(ones_col[:], ones_col_d[:])
            nc.sync.dma_start(I128[:], I128_d[:])
            nc.gpsimd.dma_start(I128bf[:], I128_d[:])
            nc.sync.dma_start(W1c_sb[:], W1c_d[:])
            nc.sync.dma_start(tvals[:], tvals_d[:])
            for g in range(NG):
                nc.sync.dma_start(yg[g][:], y_init_d[g * GCH : (g + 1) * GCH, :])
            nc.sync.dma_start(Yacc[:], Y_init_d[:, :])

            # ones row: from ones_col via I128? simpler: memset 1.0
            nc.vector.memset(ones_bf[:], 1.0)

            # ------------- init: compute -------------
            # b1tab[:, i] = b1c + t_i * W1[0, :]
            ps = pmm.tile([128, 1024], f32, tag="mm")
            nc.tensor.matmul(
                ps[:, 0:nsteps], W1c_sb[0:1, :], tvals[0:1, :], start=True, stop=True
            )
            nc.scalar.activation(
                b1tab[:], ps[:, 0:nsteps], AF.Identity, bias=b1c_sb[:, 0:1]
            )

            # W3 scaling: z-cols * sqrt(dt), q-col * dt  (cast to bf16)
            nc.vector.tensor_scalar_mul(W3_bf[:, 0:3], W3_f[:, 0:3], float(SQRT_DT))
            nc.vector.tensor_scalar_mul(W3_bf[:, 3:4], W3_f[:, 3:4], float(DT))
            # b3 scaling + replicate x8 into bf16 row [1, 32]
            nc.vector.tensor_scalar_mul(b3s[0:1, 0:3], b3f[0:1, 0:3], float(SQRT_DT))
            nc.vector.tensor_scalar_mul(b3s[0:1, 3:4], b3f[0:1, 3:4], float(DT))
            nc.vector.tensor_copy(b3rep[0:1, 0:4], b3s[0:1, :])
            nc.vector.tensor_copy(b3rep[0:1, 4:8], b3rep[0:1, 0:4])
            nc.vector.tensor_copy(b3rep[0:1, 8:16], b3rep[0:1, 0:8])
            nc.vector.tensor_copy(b3rep[0:1, 16:32], b3rep[0:1, 0:16])

            # per-quarter prepass: dd = dW - dZ ; swp = sigma0*sqrt(dt)*sum_j dW
            for q in range(NQ):
                nsq = max(0, min(nsteps, (q + 1) * QSTEPS) - q * QSTEPS)
                if nsq == 0:
                    continue
                eng = nc.vector if q % 2 == 0 else nc.gpsimd
                eng.tensor_tensor(
                    dds[q][:, 0 : nsq * 48],
                    dWs[q][:, 0 : nsq * 48],
                    dds[q][:, 0 : nsq * 48],
                    op=ALU.subtract,
                )
                lo = q * QSTEPS * 16
                src = dWs[q][:, 0 : nsq * 48].rearrange("p (s j) -> p s j", j=3)
                nc.vector.tensor_reduce(
                    swp[:, lo : lo + nsq * 16], src, axis=AX.X, op=ALU.add
                )
            nc.vector.tensor_scalar_mul(swp[:], swp[:], float(SIGMA0 * SQRT_DT))

            # initial y casts
            for g in range(NG):
                nc.vector.tensor_copy(yg_bf[g][:], yg[g][:])

            # ------------- time-step loop -------------
            for i in range(nsteps):
                qi, ri = divmod(i, QSTEPS)

                h1ps = [pmm.tile([128, 1024], f32, tag="mm", name=f"h1ps{i}_{g}")
                        for g in range(NG)]
                h2ps = [pmm.tile([128, 1024], f32, tag="mm", name=f"h2ps{i}_{g}")
                        for g in range(NG)]
                h1sb = [h1pool.tile([128, 1024], bf16, tag="h1", name=f"h1sb{i}_{g}")
                        for g in range(NG)]
                h2sb = [h2pool.tile([128, 1024], bf16, tag="h2", name=f"h2sb{i}_{g}")
                        for g in range(NG)]
                zqf_ps = pzq.tile([128, 64], f32, tag="zq", name=f"zqps{i}")

                # --- L1 (both groups): h1[f, b] via K=8 block-diag stationaries
                for g in range(NG):
                    for c in range(GCH):
                        nc.tensor.matmul(
                            h1ps[g][:, c * 128 : (c + 1) * 128],
                            L1bg_bf[:, c * 128 : (c + 1) * 128],
                            yg_bf[g][:],
                            start=True,
                            stop=True,
                        )

                # --- relu1 (+ per-step bias) -> bf16 (slices: ACT/DVE)
                # engine map over (g, slice): relu1: [ACT, DVE], [ACT, ACT]
                r1eng = [[nc.scalar, nc.vector], [nc.scalar, nc.scalar]]
                r2eng = [[nc.vector, nc.scalar], [nc.scalar, nc.vector]]
                for g in range(NG):
                    for k in range(2):
                        eng = r1eng[g][k]
                        dst = h1sb[g][:, k * 512 : (k + 1) * 512]
                        src = h1ps[g][:, k * 512 : (k + 1) * 512]
                        if eng is nc.scalar:
                            eng.activation(dst, src, AF.Relu, bias=b1tab[:, i : i + 1])
                        else:
                            eng.tensor_scalar(
                                dst, src, b1tab[:, i : i + 1], 0.0,
                                op0=ALU.add, op1=ALU.max,
                            )

                # --- L2
                for g in range(NG):
                    for k in range(2):
                        nc.tensor.matmul(
                            h2ps[g][:, k * 512 : (k + 1) * 512],
                            W2bd_bf[:],
                            h1sb[g][:, k * 512 : (k + 1) * 512],
                            start=True,
                            stop=True,
                        )

                # --- relu2 -> bf16
                for g in range(NG):
                    for k in range(2):
                        eng = r2eng[g][k]
                        dst = h2sb[g][:, k * 512 : (k + 1) * 512]
                        src = h2ps[g][:, k * 512 : (k + 1) * 512]
                        if eng is nc.scalar:
                            eng.activation(dst, src, AF.Relu, bias=b2c_sb[:, 0:1])
                        else:
                            eng.tensor_scalar(
                                dst, src, b2c_sb[:, 0:1], 0.0,
                                op0=ALU.add, op1=ALU.max,
                            )

                # --- L3: zq folded [p, g*32 + c*4 + m], accumulated with bias
                for g in range(NG):
                    nc.tensor.matmul(
                        zqf_ps[:, g * 32 : (g + 1) * 32], ones_bf[0:1, :],
                        b3rep[0:1, :], start=True, stop=False,
                        skip_group_check=True,
                    )
                    for c in range(GCH):
                        nc.tensor.matmul(
                            zqf_ps[:, g * 32 + c * 4 : g * 32 + (c + 1) * 4],
                            h2sb[g][:, c * 128 : (c + 1) * 128],
                            W3_bf[:],
                            start=False,
                            stop=(c == GCH - 1),
                            skip_group_check=True,
                        )

                # --- epilogue ---
                zqf_sb = epool.tile([128, 64], f32, tag="zqf", name=f"zqf{i}")
                zz = epool.tile([128, 96], f32, tag="zz", name=f"zz{i}")
                uurj = epool.tile([128, 32], f32, tag="uurj", name=f"uurj{i}")
                rrq = epool.tile([128, 32], f32, tag="rrq", name=f"rrq{i}")
                uf = epool.tile([128, 16], f32, tag="uf", name=f"uf{i}")
                incr = [epool.tile([128, GCH], bf16, tag=f"incr{g}", name=f"incr{i}_{g}")
                        for g in range(NG)]

                # PSUM -> SBUF copies (A early on ACT, B on DVE)
                nc.scalar.activation(zqf_sb[:, 0:32], zqf_ps[:, 0:32], AF.Copy)
                nc.vector.tensor_copy(zqf_sb[:, 32:64], zqf_ps[:, 32:64])

                zq4 = zqf_sb[:].rearrange("p (g c m) -> p g c m", g=2, m=4)
                qview = zq4[:, :, :, 3:4]

                # y-critical path: incr = q*dt + swp ; y += incr^T ; cast
                for g in range(NG):
                    sweepv = swp[:, i * 16 + g * 8 : i * 16 + g * 8 + 8]
                    nc.gpsimd.tensor_tensor(
                        incr[g][:].rearrange("p (c o) -> p c o", o=1),
                        zq4[:, g, :, 3:4],
                        sweepv.rearrange("p (c o) -> p c o", o=1),
                        op=ALU.add,
                    )
                    incr_t = ptr.tile([GCH, 128], bf16, tag="tr", name=f"tr{i}_{g}")
                    nc.tensor.transpose(incr_t[:], incr[g][:], I128bf[:])
                    nc.vector.tensor_tensor(yg[g][:], yg[g][:], incr_t[:], op=ALU.add)
                    nc.gpsimd.tensor_copy(yg_bf[g][:], yg[g][:])

                # slack path: residual + Y accumulation + loss
                base = ri * 48
                nsl = [dWs[qi][:, base : base + 24], dds[qi][:, base : base + 24],
                       dWs[qi][:, base + 24 : base + 48], dds[qi][:, base + 24 : base + 48]]
                for g in range(NG):
                    zv = zq4[:, g, :, 0:3]
                    nc.gpsimd.tensor_tensor(
                        zz[:, g * 48 : g * 48 + 24].rearrange("p (c j) -> p c j", j=3),
                        zv, nsl[2 * g][:].rearrange("p (c j) -> p c j", j=3), op=ALU.mult)
                    nc.gpsimd.tensor_tensor(
                        zz[:, g * 48 + 24 : g * 48 + 48].rearrange("p (c j) -> p c j", j=3),
                        zv, nsl[2 * g + 1][:].rearrange("p (c j) -> p c j", j=3), op=ALU.mult)
                # uurj = [uA(8) rA(8) uB(8) rB(8)]
                nc.vector.tensor_reduce(
                    uurj[:].rearrange("p (h o) -> p h o", o=1),
                    zz[:].rearrange("p (h j) -> p h j", j=3),
                    axis=AX.X, op=ALU.add,
                )
                u4 = uurj[:].rearrange("p (g t c) -> p g t c", g=2, t=2)
                # rrq = [rr(16) | qq(16)]
                nc.gpsimd.tensor_tensor(
                    rrq[:, 0:16].rearrange("p (g c) -> p g c", g=2),
                    u4[:, :, 1, :], u4[:, :, 1, :], op=ALU.mult)
                nc.gpsimd.tensor_tensor(
                    rrq[:, 16:32].rearrange("p (g c o) -> p g c o", g=2, o=1),
                    qview, qview, op=ALU.mult)
                nc.gpsimd.tensor_scalar_mul(rrq[:, 16:32], rrq[:, 16:32], SC_F * SC_F)
                nc.gpsimd.tensor_tensor(
                    uf[:].rearrange("p (g c) -> p g c", g=2),
                    u4[:, :, 0, :],
                    rrq[:, 16:32].rearrange("p (g c) -> p g c", g=2),
                    op=ALU.subtract)
                nc.gpsimd.tensor_tensor(Yacc[:], Yacc[:], uf[:], op=ALU.add)
                nc.tensor.matmul(
                    loss_ps[:], ones_col[:], rrq[:, 0:16],
                    start=(i == 0), stop=False, skip_group_check=True,
                )

            # ------------- terminal loss -------------
            for g in range(NG):
                nc.scalar.activation(ysq[g][:], yg[g][:], AF.Square)
                ysq_ps = pzq.tile([128, 64], f32, tag="zq", name=f"term{g}")
                nc.tensor.matmul(
                    ysq_ps[:, 0:GCH], ysq[g][:], I128[0:GCH, 0:GCH], is_transpose=True
                )
                nc.vector.tensor_tensor(
                    ee[g][:], Yacc[:, g * GCH : (g + 1) * GCH], ysq_ps[:, 0:GCH],
                    op=ALU.subtract)
                nc.scalar.activation(ee[g][:], ee[g][:], AF.Square)
                nc.tensor.matmul(
                    loss_ps[0:1, g * 8 : g * 8 + 8],
                    ones_col[:],
                    ee[g][:],
                    start=False,
                    stop=(g == NG - 1),
                    skip_group_check=True,
                )
            nc.vector.tensor_copy(loss_sb[:], loss_ps[:])
            nc.vector.tensor_reduce(
                loss1[:],
                loss_sb[0:1, :].rearrange("p (o c) -> p o c", o=1),
                axis=AX.X,
                op=ALU.add,
            )
            nc.vector.tensor_scalar_mul(loss1[:], loss1[:], 1.0 / B)
            nc.sync.dma_start(loss_out[:], loss1[:])
            if debug:
                for g in range(NG):
                    nc.sync.dma_start(y_out[g * GCH : (g + 1) * GCH, :], yg[g][:])
                    nc.sync.dma_start(Y_out[:, g * GCH : (g + 1) * GCH], Yacc[:, g * GCH : (g + 1) * GCH])

    nc.compile()
    return nc


def _host_inputs(nsteps, y0, Y0, zW1, zb1, zW2, zb2, zW3, zb3, qW1, qb1, qW2, qb2, qW3, qb3, dW, dZ):
    """Per-core input maps. Layout/slicing only — no arithmetic on inputs."""
    f = np.float32
    QSTEPS = (nsteps + NQ - 1) // NQ
    W1row1 = np.concatenate([zW1[1], qW1[1]]).astype(f)
    L1bg = np.zeros((GCH, GCH * 128), f)
    for c in range(GCH):
        L1bg[c, c * 128 : (c + 1) * 128] = W1row1
    W1c = np.concatenate([zW1, qW1], axis=1).astype(f)  # (2,128)
    W2bd = np.zeros((128, 128), f)
    W2bd[0:64, 0:64] = zW2
    W2bd[64:128, 64:128] = qW2
    W3c = np.zeros((128, 4), f)
    W3c[0:64, 0:3] = zW3
    W3c[64:128, 3] = qW3[:, 0]
    b1c = np.concatenate([zb1, qb1]).astype(f).reshape(128, 1)
    b2c = np.concatenate([zb2, qb2]).astype(f).reshape(128, 1)
    b3c = np.concatenate([zb3, qb3]).astype(f).reshape(1, 4)
    tvals = (np.arange(nsteps) * DT).astype(f).reshape(1, nsteps)
    ones_col = np.ones((128, 1), f)
    I128 = np.eye(128, dtype=f)
    y_init = np.broadcast_to(np.asarray(y0, f).reshape(1, 1), (16, 128)).copy()
    Y_init = np.broadcast_to(np.asarray(Y0, f).reshape(1, 1), (128, 16)).copy()

    shared = dict(
        L1bg=L1bg, W1c=W1c, W2bd=W2bd, W3c=W3c, b1c=b1c, b2c=b2c, b3c=b3c,
        tvals=tvals, ones_col=ones_col, I128=I128,
        y_init=y_init, Y_init=Y_init,
    )

    in_maps = []
    for core in range(NCORES):
        o = core * BC
        m = dict(shared)
        for name, arr in (("dWf", dW), ("dZf", dZ)):
            # fold: [nsteps, 2048, 3] -> [128, nsteps*48],
            # col = i*48 + c*3 + j, path = c*128 + p
            x = np.ascontiguousarray(arr[:nsteps, o : o + BC, :]).astype(f)
            x = x.reshape(nsteps, NCH, 128, 3).transpose(2, 0, 1, 3)
            x = np.ascontiguousarray(x).reshape(128, nsteps * 48)
            for q in range(NQ):
                sl = x[:, q * QSTEPS * 48 : (q + 1) * QSTEPS * 48]
                buf = np.zeros((128, QSTEPS * 48), f)
                buf[:, : sl.shape[1]] = sl
                m[f"{name}{q}"] = buf
        in_maps.append(m)
    return in_maps


def _run(nsteps, inputs, debug=False):
    global LAST_EXEC_NS, LAST_RESULTS
    from concourse import bass_utils

    key = (nsteps, debug)
    if key not in _CACHE:
        _CACHE[key] = _build(nsteps, debug=debug)
    nc = _CACHE[key]

    in_maps = _host_inputs(nsteps, **inputs)
    trace = bool(os.environ.get("BASS_TRACE"))
    kwargs = {}
    if trace:
        import tempfile

        kwargs = dict(trace=True, tmpdir=tempfile.mkdtemp(prefix="bsde_trace_"))
    res = bass_utils.run_bass_kernel_spmd(
        nc, in_maps, core_ids=list(range(NCORES)), **kwargs
    )
    LAST_RESULTS = res
    LAST_EXEC_NS = res.exec_time_ns
    return res


def kernel(**inputs):
    inputs = {k: np.asarray(v, np.float32) for k, v in inputs.items()}
    res = _run(NSTEPS, inputs, debug=False)
    total = np.float32(0.0)
    for core in range(NCORES):
        total += res.results[core]["loss_out"][0, 0]
    return np.array(total, dtype=np.float32)


# revision 7
# speedup vs baseline: 1.1084x; 1.1084x over previous
"""Trainium2 Bass kernel for the DeepBSDE loss (nn_BaseDeepBSDE).

Data-parallel over 8 NeuronCores: each core simulates 2048 Monte-Carlo
paths through the 100-step SDE loop and produces a partial loss sum;
the host sums the 8 partial scalars.

v2 design (vs v1 baseline):
  - Two software-pipelined path groups (A: chunks 0-7, B: chunks 8-15,
    1024 paths each). Group B's matmuls overlap group A's epilogue so
    the PE stays dense (ramps to full 2.4 GHz pstate instead of 1.2).
  - L1 as K=1 rank-1 matmuls straight from y rows (no block-diag L1b).
  - dd = dW - dZ precomputed once per quarter (residual = z . dd),
    dropping the v-branch of the epilogue.
  - relu work split across Scalar/Vector/GpSimd engines.
  - bf16 PE transpose for the y-update increment.
"""

import os
import sys

sys.path.insert(0, "/opt/trn_rl_repo")

import numpy as np

B = 16384
NSTEPS = 100
DIMW = 3
DT = 0.01
SQRT_DT = DT**0.5
SIGMA0 = 0.5
NCORES = 8
BC = B // NCORES  # 2048 paths per core
NCH = BC // 128  # 16 chunks of 128 paths
NG = 2  # path groups per core
GCH = NCH // NG  # 8 chunks per group
NQ = 4  # noise quarters

LAST_EXEC_NS = None
LAST_RESULTS = None

_CACHE = {}


def _build(nsteps, debug=False):
    import concourse.tile as tile
    from concourse import bacc, mybir

    f32 = mybir.dt.float32
    bf16 = mybir.dt.bfloat16
    AF = mybir.ActivationFunctionType
    ALU = mybir.AluOpType
    AX = mybir.AxisListType

    nc = bacc.Bacc("TRN2", target_bir_lowering=False, debug=False, num_devices=NCORES)

    QSTEPS = (nsteps + NQ - 1) // NQ
    dWf_d = [
        nc.dram_tensor(f"dWf{q}", [128, QSTEPS * 48], f32, kind="ExternalInput").ap()
        for q in range(NQ)
    ]
    dZf_d = [
        nc.dram_tensor(f"dZf{q}", [128, QSTEPS * 48], f32, kind="ExternalInput").ap()
        for q in range(NQ)
    ]
    L1bg_d = nc.dram_tensor("L1bg", [GCH, GCH * 128], f32, kind="ExternalInput").ap()
    W1c_d = nc.dram_tensor("W1c", [2, 128], f32, kind="ExternalInput").ap()
    W2bd_d = nc.dram_tensor("W2bd", [128, 128], f32, kind="ExternalInput").ap()
    W3c_d = nc.dram_tensor("W3c", [128, 4], f32, kind="ExternalInput").ap()
    b1c_d = nc.dram_tensor("b1c", [128, 1], f32, kind="ExternalInput").ap()
    b2c_d = nc.dram_tensor("b2c", [128, 1], f32, kind="ExternalInput").ap()
    b3c_d = nc.dram_tensor("b3c", [1, 4], f32, kind="ExternalInput").ap()
    tvals_d = nc.dram_tensor("tvals", [1, nsteps], f32, kind="ExternalInput").ap()
    ones_col_d = nc.dram_tensor("ones_col", [128, 1], f32, kind="ExternalInput").ap()
    I128_d = nc.dram_tensor("I128", [128, 128], f32, kind="ExternalInput").ap()
    y_init_d = nc.dram_tensor("y_init", [16, 128], f32, kind="ExternalInput").ap()
    Y_init_d = nc.dram_tensor("Y_init", [128, 16], f32, kind="ExternalInput").ap()

    loss_out = nc.dram_tensor("loss_out", [1, 1], f32, kind="ExternalOutput").ap()
    if debug:
        y_out = nc.dram_tensor("y_out", [16, 128], f32, kind="ExternalOutput").ap()
        Y_out = nc.dram_tensor("Y_out", [128, 16], f32, kind="ExternalOutput").ap()
        zq_out = nc.dram_tensor("zq_out", [128, 64], f32, kind="ExternalOutput").ap()

    SC_F = float((0.5 / DT) ** 0.5)  # (SC_F * qDT)^2 = 0.5*dt*q^2

    with tile.TileContext(nc) as tc:
        from contextlib import ExitStack

        with ExitStack() as ctx:
            cpool = ctx.enter_context(tc.tile_pool(name="const", bufs=1))
            h1pool = ctx.enter_context(tc.tile_pool(name="h1sb", bufs=3))
            h2pool = ctx.enter_context(tc.tile_pool(name="h2sb", bufs=3))
            epool = ctx.enter_context(tc.tile_pool(name="epil", bufs=3))
            pmm = ctx.enter_context(tc.tile_pool(name="pmm", bufs=2, space="PSUM"))
            pzq = ctx.enter_context(tc.tile_pool(name="pzq", bufs=1, space="PSUM"))
            ptr = ctx.enter_context(tc.tile_pool(name="ptr", bufs=2, space="PSUM"))
            ploss = ctx.enter_context(tc.tile_pool(name="ploss", bufs=1, space="PSUM"))

            # ------------- persistent SBUF tiles -------------
            dWs = [cpool.tile([128, QSTEPS * 48], f32, tag=f"dw{q}", name=f"dws{q}") for q in range(NQ)]
            dds = [cpool.tile([128, QSTEPS * 48], f32, tag=f"dz{q}", name=f"dds{q}") for q in range(NQ)]
            swp = cpool.tile([128, nsteps * 16], f32, tag="swp")
            W2bd_bf = cpool.tile([128, 128], bf16, tag="w2bd")
            L1bg_bf = cpool.tile([GCH, GCH * 128], bf16, tag="l1bg")
            W3_bf = cpool.tile([128, 4], bf16, tag="w3")
            W3_f = cpool.tile([128, 4], f32, tag="w3f")
            b1tab = cpool.tile([128, nsteps], f32, tag="b1tab")
            b1c_sb = cpool.tile([128, 1], f32, tag="b1c")
            b2c_sb = cpool.tile([128, 1], f32, tag="b2c")
            b3s = cpool.tile([1, 4], f32, tag="b3s")
            b3f = cpool.tile([1, 4], f32, tag="b3f")
            b3rep = cpool.tile([1, 64], bf16, tag="b3rep")
            ones_bf = cpool.tile([1, 128], bf16, tag="ones_bf")
            ones_col = cpool.tile([128, 1], f32, tag="ones_col")
            I128 = cpool.tile([128, 128], f32, tag="i128")
            I128bf = cpool.tile([128, 128], bf16, tag="i128bf")
            W1c_sb = cpool.tile([2, 128], f32, tag="w1c")
            tvals = cpool.tile([1, nsteps], f32, tag="tvals")
            yg_bf = [cpool.tile([GCH, 128], bf16, tag=f"ybf{g}", name=f"ygbf{g}") for g in range(NG)]
            Yacc = cpool.tile([128, 16], f32, tag="Yacc")
            ysq = [cpool.tile([GCH, 128], f32, tag=f"ysq{g}", name=f"ysq{g}") for g in range(NG)]
            ee = [cpool.tile([128, GCH], f32, tag=f"ee{g}", name=f"ee{g}") for g in range(NG)]
            loss_sb = cpool.tile([1, 16], f32, tag="loss_sb")
            loss1 = cpool.tile([1, 1], f32, tag="loss1")

            loss_ps = ploss.tile([1, 16], f32, tag="loss")

            # ------------- init: DMAs -------------
            for q in range(NQ):
                nc.sync.dma_start(dWs[q][:], dWf_d[q][:])
                nc.sync.dma_start(dds[q][:], dZf_d[q][:])
            nc.gpsimd.dma_start(W2bd_bf[:], W2bd_d[:])
            nc.gpsimd.dma_start(L1bg_bf[:], L1bg_d[:])
            nc.sync.dma_start(W3_f[:], W3c_d[:])
            nc.sync.dma_start(b1c_sb[:], b1c_d[:])
            nc.sync.dma_start(b2c_sb[:], b2c_d[:])
            nc.sync.dma_start(b3f[:], b3c_d[:])
            nc.sync.dma_start(ones_col[:], ones_col_d[:])
            nc.sync.dma_start(I128[:], I128_d[:])
            nc.gpsimd.dma_start(I128bf[:], I128_d[:])
            nc.sync.dma_start(W1c_sb[:], W1c_d[:])
            nc.sync.dma_start(tvals[:], tvals_d[:])
            for g in range(NG):
                nc.gpsimd.dma_start(yg_bf[g][:], y_init_d[g * GCH : (g + 1) * GCH, :])
            nc.sync.dma_start(Yacc[:], Y_init_d[:, :])

            # ones row: from ones_col via I128? simpler: memset 1.0
            nc.vector.memset(ones_bf[:], 1.0)

            # ------------- init: compute -------------
            # b1tab[:, i] = b1c + t_i * W1[0, :]
            ps = pmm.tile([128, 1024], f32, tag="mm")
            nc.tensor.matmul(
                ps[:, 0:nsteps], W1c_sb[0:1, :], tvals[0:1, :], start=True, stop=True
            )
            nc.scalar.activation(
                b1tab[:], ps[:, 0:nsteps], AF.Identity, bias=b1c_sb[:, 0:1]
            )

            # W3 scaling: z-cols * sqrt(dt), q-col * dt  (cast to bf16)
            nc.vector.tensor_scalar_mul(W3_bf[:, 0:3], W3_f[:, 0:3], float(SQRT_DT))
            nc.vector.tensor_scalar_mul(W3_bf[:, 3:4], W3_f[:, 3:4], float(DT))
            # b3 scaling + replicate x8 into bf16 row [1, 32]
            nc.vector.tensor_scalar_mul(b3s[0:1, 0:3], b3f[0:1, 0:3], float(SQRT_DT))
            nc.vector.tensor_scalar_mul(b3s[0:1, 3:4], b3f[0:1, 3:4], float(DT))
            nc.vector.tensor_copy(b3rep[0:1, 0:4], b3s[0:1, :])
            nc.vector.tensor_copy(b3rep[0:1, 4:8], b3rep[0:1, 0:4])
            nc.vector.tensor_copy(b3rep[0:1, 8:16], b3rep[0:1, 0:8])
            nc.vector.tensor_copy(b3rep[0:1, 16:32], b3rep[0:1, 0:16])
            nc.vector.tensor_copy(b3rep[0:1, 32:64], b3rep[0:1, 0:32])

            # per-quarter prepass: dd = dW - dZ ; swp = sigma0*sqrt(dt)*sum_j dW
            for q in range(NQ):
                nsq = max(0, min(nsteps, (q + 1) * QSTEPS) - q * QSTEPS)
                if nsq == 0:
                    continue
                eng = nc.vector if q % 2 == 0 else nc.gpsimd
                eng.tensor_tensor(
                    dds[q][:, 0 : nsq * 48],
                    dWs[q][:, 0 : nsq * 48],
                    dds[q][:, 0 : nsq * 48],
                    op=ALU.subtract,
                )
                lo = q * QSTEPS * 16
                src = dWs[q][:, 0 : nsq * 48].rearrange("p (s j) -> p s j", j=3)
                nc.vector.tensor_reduce(
                    swp[:, lo : lo + nsq * 16], src, axis=AX.X, op=ALU.add
                )
            nc.vector.tensor_scalar_mul(swp[:], swp[:], float(SIGMA0 * SQRT_DT))

            # ------------- time-step loop -------------
            for i in range(nsteps):
                qi, ri = divmod(i, QSTEPS)

                h1ps = [pmm.tile([128, 1024], f32, tag="mm", name=f"h1ps{i}_{g}")
                        for g in range(NG)]
                h2ps = [pmm.tile([128, 1024], f32, tag="mm", name=f"h2ps{i}_{g}")
                        for g in range(NG)]
                h1sb = [h1pool.tile([128, 1024], bf16, tag="h1", name=f"h1sb{i}_{g}")
                        for g in range(NG)]
                h2sb = [h2pool.tile([128, 1024], bf16, tag="h2", name=f"h2sb{i}_{g}")
                        for g in range(NG)]
                zqf_ps = pzq.tile([128, 64], f32, tag="zq", name=f"zqps{i}")

                # --- L1 (both groups): h1[f, b] via K=8 block-diag stationaries
                for g in range(NG):
                    for c in range(GCH):
                        nc.tensor.matmul(
                            h1ps[g][:, c * 128 : (c + 1) * 128],
                            L1bg_bf[:, c * 128 : (c + 1) * 128],
                            yg_bf[g][:],
                            start=True,
                            stop=True,
                        )

                # --- relu1 (+ per-step bias) -> bf16 (slices: ACT/DVE)
                # engine map over (g, slice): relu1: [ACT, DVE], [ACT, ACT]
                r1eng = [[nc.scalar, nc.vector], [nc.scalar, nc.scalar]]
                r2eng = [[nc.vector, nc.scalar], [nc.scalar, nc.vector]]
                for g in range(NG):
                    for k in range(2):
                        eng = r1eng[g][k]
                        dst = h1sb[g][:, k * 512 : (k + 1) * 512]
                        src = h1ps[g][:, k * 512 : (k + 1) * 512]
                        if eng is nc.scalar:
                            eng.activation(dst, src, AF.Relu, bias=b1tab[:, i : i + 1])
                        else:
                            eng.tensor_scalar(
                                dst, src, b1tab[:, i : i + 1], 0.0,
                                op0=ALU.add, op1=ALU.max,
                            )

                # --- L2
                for g in range(NG):
                    for k in range(2):
                        nc.tensor.matmul(
                            h2ps[g][:, k * 512 : (k + 1) * 512],
                            W2bd_bf[:],
                            h1sb[g][:, k * 512 : (k + 1) * 512],
                            start=True,
                            stop=True,
                        )

                # --- relu2 -> bf16
                for g in range(NG):
                    for k in range(2):
                        eng = r2eng[g][k]
                        dst = h2sb[g][:, k * 512 : (k + 1) * 512]
                        src = h2ps[g][:, k * 512 : (k + 1) * 512]
                        if eng is nc.scalar:
                            eng.activation(dst, src, AF.Relu, bias=b2c_sb[:, 0:1])
                        else:
                            eng.tensor_scalar(
                                dst, src, b2c_sb[:, 0:1], 0.0,
                                op0=ALU.add, op1=ALU.max,
                            )

                # --- L3: zq folded [p, g*32 + c*4 + m], accumulated with bias
                nc.tensor.matmul(
                    zqf_ps[:], ones_bf[0:1, :],
                    b3rep[0:1, :], start=True, stop=False,
                    skip_group_check=True,
                )
                for g in range(NG):
                    for c in range(GCH):
                        nc.tensor.matmul(
                            zqf_ps[:, g * 32 + c * 4 : g * 32 + (c + 1) * 4],
                            h2sb[g][:, c * 128 : (c + 1) * 128],
                            W3_bf[:],
                            start=False,
                            stop=(c == GCH - 1),
                            skip_group_check=True,
                        )

                # --- epilogue ---
                zqf_sb = epool.tile([128, 64], f32, tag="zqf", name=f"zqf{i}")
                zz = epool.tile([128, 96], f32, tag="zz", name=f"zz{i}")
                uurj = epool.tile([128, 32], f32, tag="uurj", name=f"uurj{i}")
                rrq = epool.tile([128, 32], f32, tag="rrq", name=f"rrq{i}")
                uf = epool.tile([128, 16], f32, tag="uf", name=f"uf{i}")
                incr = [epool.tile([128, GCH], bf16, tag=f"incr{g}", name=f"incr{i}_{g}")
                        for g in range(NG)]

                # PSUM -> SBUF copies (A early on ACT, B on DVE)
                nc.scalar.activation(zqf_sb[:, 0:32], zqf_ps[:, 0:32], AF.Copy)
                nc.vector.tensor_copy(zqf_sb[:, 32:64], zqf_ps[:, 32:64])

                zq4 = zqf_sb[:].rearrange("p (g c m) -> p g c m", g=2, m=4)
                qview = zq4[:, :, :, 3:4]

                # y-critical path: incr = q*dt + swp ; y_bf += incr^T
                for g in range(NG):
                    sweepv = swp[:, i * 16 + g * 8 : i * 16 + g * 8 + 8]
                    nc.vector.tensor_tensor(
                        incr[g][:].rearrange("p (c o) -> p c o", o=1),
                        zq4[:, g, :, 3:4],
                        sweepv.rearrange("p (c o) -> p c o", o=1),
                        op=ALU.add,
                    )
                    incr_t = ptr.tile([GCH, 128], bf16, tag="tr", name=f"tr{i}_{g}")
                    nc.tensor.transpose(incr_t[:], incr[g][:], I128bf[:])
                    nc.vector.tensor_tensor(yg_bf[g][:], yg_bf[g][:], incr_t[:], op=ALU.add)

                # slack path: residual + Y accumulation + loss
                base = ri * 48
                zvall = zq4[:, :, :, 0:3]
                nc.gpsimd.tensor_tensor(
                    zz[:, 0:48].rearrange("p (g c j) -> p g c j", g=2, j=3),
                    zvall, dWs[qi][:, base : base + 48].rearrange(
                        "p (g c j) -> p g c j", g=2, j=3), op=ALU.mult)
                nc.gpsimd.tensor_tensor(
                    zz[:, 48:96].rearrange("p (g c j) -> p g c j", g=2, j=3),
                    zvall, dds[qi][:, base : base + 48].rearrange(
                        "p (g c j) -> p g c j", g=2, j=3), op=ALU.mult)
                # uurj = [uA(8) uB(8) rA(8) rB(8)]
                nc.vector.tensor_reduce(
                    uurj[:].rearrange("p (h o) -> p h o", o=1),
                    zz[:].rearrange("p (h j) -> p h j", j=3),
                    axis=AX.X, op=ALU.add,
                )
                u4 = uurj[:].rearrange("p (t g c) -> p t g c", t=2, g=2)
                # rrq = [rr(16) | qq(16)]
                nc.gpsimd.tensor_tensor(
                    rrq[:, 0:16].rearrange("p (g c) -> p g c", g=2),
                    u4[:, 1, :, :], u4[:, 1, :, :], op=ALU.mult)
                nc.gpsimd.tensor_tensor(
                    rrq[:, 16:32].rearrange("p (g c o) -> p g c o", g=2, o=1),
                    qview, qview, op=ALU.mult)
                nc.gpsimd.tensor_scalar_mul(rrq[:, 16:32], rrq[:, 16:32], SC_F * SC_F)
                nc.gpsimd.tensor_tensor(
                    uf[:].rearrange("p (g c) -> p g c", g=2),
                    u4[:, 0, :, :],
                    rrq[:, 16:32].rearrange("p (g c) -> p g c", g=2),
                    op=ALU.subtract)
                nc.gpsimd.tensor_tensor(Yacc[:], Yacc[:], uf[:], op=ALU.add)
                nc.tensor.matmul(
                    loss_ps[:], ones_col[:], rrq[:, 0:16],
                    start=(i == 0), stop=False, skip_group_check=True,
                )

            # ------------- terminal loss -------------
            for g in range(NG):
                nc.scalar.activation(ysq[g][:], yg_bf[g][:], AF.Square)
                ysq_ps = pzq.tile([128, 64], f32, tag="zq", name=f"term{g}")
                nc.tensor.matmul(
                    ysq_ps[:, 0:GCH], ysq[g][:], I128[0:GCH, 0:GCH], is_transpose=True
                )
                nc.vector.tensor_tensor(
                    ee[g][:], Yacc[:, g * GCH : (g + 1) * GCH], ysq_ps[:, 0:GCH],
                    op=ALU.subtract)
                nc.scalar.activation(ee[g][:], ee[g][:], AF.Square)
                nc.tensor.matmul(
                    loss_ps[0:1, g * 8 : g * 8 + 8],
                    ones_col[:],
                    ee[g][:],
                    start=False,
                    stop=(g == NG - 1),
                    skip_group_check=True,
                )
            nc.vector.tensor_copy(loss_sb[:], loss_ps[:])
            nc.vector.tensor_reduce(
                loss1[:],
                loss_sb[0:1, :].rearrange("p (o c) -> p o c", o=1),
                axis=AX.X,
                op=ALU.add,
            )
            nc.vector.tensor_scalar_mul(loss1[:], loss1[:], 1.0 / B)
            nc.sync.dma_start(loss_out[:], loss1[:])
            if debug:
                for g in range(NG):
                    nc.sync.dma_start(y_out[g * GCH : (g + 1) * GCH, :], yg_bf[g][:])
                    nc.sync.dma_start(Y_out[:, g * GCH : (g + 1) * GCH], Yacc[:, g * GCH : (g + 1) * GCH])

    nc.compile()
    return nc


def _host_inputs(nsteps, y0, Y0, zW1, zb1, zW2, zb2, zW3, zb3, qW1, qb1, qW2, qb2, qW3, qb3, dW, dZ):
    """Per-core input maps. Layout/slicing only — no arithmetic on inputs."""
    f = np.float32
    QSTEPS = (nsteps + NQ - 1) // NQ
    W1row1 = np.concatenate([zW1[1], qW1[1]]).astype(f)
    L1bg = np.zeros((GCH, GCH * 128), f)
    for c in range(GCH):
        L1bg[c, c * 128 : (c + 1) * 128] = W1row1
    W1c = np.concatenate([zW1, qW1], axis=1).astype(f)  # (2,128)
    W2bd = np.zeros((128, 128), f)
    W2bd[0:64, 0:64] = zW2
    W2bd[64:128, 64:128] = qW2
    W3c = np.zeros((128, 4), f)
    W3c[0:64, 0:3] = zW3
    W3c[64:128, 3] = qW3[:, 0]
    b1c = np.concatenate([zb1, qb1]).astype(f).reshape(128, 1)
    b2c = np.concatenate([zb2, qb2]).astype(f).reshape(128, 1)
    b3c = np.concatenate([zb3, qb3]).astype(f).reshape(1, 4)
    tvals = (np.arange(nsteps) * DT).astype(f).reshape(1, nsteps)
    ones_col = np.ones((128, 1), f)
    I128 = np.eye(128, dtype=f)
    y_init = np.broadcast_to(np.asarray(y0, f).reshape(1, 1), (16, 128)).copy()
    Y_init = np.broadcast_to(np.asarray(Y0, f).reshape(1, 1), (128, 16)).copy()

    shared = dict(
        L1bg=L1bg, W1c=W1c, W2bd=W2bd, W3c=W3c, b1c=b1c, b2c=b2c, b3c=b3c,
        tvals=tvals, ones_col=ones_col, I128=I128,
        y_init=y_init, Y_init=Y_init,
    )

    in_maps = []
    for core in range(NCORES):
        o = core * BC
        m = dict(shared)
        for name, arr in (("dWf", dW), ("dZf", dZ)):
            # fold: [nsteps, 2048, 3] -> [128, nsteps*48],
            # col = i*48 + c*3 + j, path = c*128 + p
            x = np.ascontiguousarray(arr[:nsteps, o : o + BC, :]).astype(f)
            x = x.reshape(nsteps, NCH, 128, 3).transpose(2, 0, 1, 3)
            x = np.ascontiguousarray(x).reshape(128, nsteps * 48)
            for q in range(NQ):
                sl = x[:, q * QSTEPS * 48 : (q + 1) * QSTEPS * 48]
                buf = np.zeros((128, QSTEPS * 48), f)
                buf[:, : sl.shape[1]] = sl
                m[f"{name}{q}"] = buf
        in_maps.append(m)
    return in_maps


def _run(nsteps, inputs, debug=False):
    global LAST_EXEC_NS, LAST_RESULTS
    from concourse import bass_utils

    key = (nsteps, debug)
    if key not in _CACHE:
        _CACHE[key] = _build(nsteps, debug=debug)
    nc = _CACHE[key]

    in_maps = _host_inputs(nsteps, **inputs)
    trace = bool(os.environ.get("BASS_TRACE"))
    kwargs = {}
    if trace:
        import tempfile

        kwargs = dict(trace=True, tmpdir=tempfile.mkdtemp(prefix="bsde_trace_"))
    res = bass_utils.run_bass_kernel_spmd(
        nc, in_maps, core_ids=list(range(NCORES)), **kwargs
    )
    LAST_RESULTS = res
    LAST_EXEC_NS = res.exec_time_ns
    return res


def kernel(**inputs):
    inputs = {k: np.asarray(v, np.float32) for k, v in inputs.items()}
    res = _run(NSTEPS, inputs, debug=False)
    total = np.float32(0.0)
    for core in range(NCORES):
        total += res.results[core]["loss_out"][0, 0]
    return np.array(total, dtype=np.float32)


# revision 10
# speedup vs baseline: 1.2865x; 1.1607x over previous
"""Trainium2 Bass kernel for the DeepBSDE loss (nn_BaseDeepBSDE).

Data-parallel over 8 NeuronCores: each core simulates 2048 Monte-Carlo
paths through the 100-step SDE loop and produces a partial loss sum;
the host sums the 8 partial scalars.

v2d design:
  - Two path groups (A: chunks 0-7, B: chunks 8-15) emitted as
    anti-phase rounds: group B's matmuls overlap group A's epilogue so
    the PE stays dense and ramps to the full 2.4 GHz pstate.
  - L1 as K=8 block-diag matmuls from the y row tile.
  - L3 emits 5 columns per chunk: z0..z2*sqrt(dt), q*dt, q*dt*SC_F —
    the extra pre-scaled q column makes fdt a single multiply.
  - swp (sigma*sqrt(dt)*sum_j dW) is pre-padded into the zq column
    layout, so the PSUM->SBUF copy IS the y-increment add.
  - dd = dW - dZ precomputed per quarter (residual = z . dd).
  - loss accumulated via DVE tensor_tensor_reduce chain (no PSUM bank).
  - y kept in bf16 only; bf16 PE transpose for the y update.
"""

import os
import sys

sys.path.insert(0, "/opt/trn_rl_repo")

import numpy as np

B = 16384
NSTEPS = 100
DT = 0.01
SQRT_DT = DT**0.5
SIGMA0 = 0.5
NCORES = 8
BC = B // NCORES  # 2048 paths per core
NCH = BC // 128  # 16 chunks of 128 paths
NG = 2
GCH = NCH // NG  # 8 chunks per group
NQ = 4
M5 = 5  # columns per chunk in zq layout

LAST_EXEC_NS = None
LAST_RESULTS = None

_CACHE = {}


def _build(nsteps, debug=False):
    import concourse.tile as tile
    from concourse import bacc, mybir

    f32 = mybir.dt.float32
    bf16 = mybir.dt.bfloat16
    AF = mybir.ActivationFunctionType
    ALU = mybir.AluOpType
    AX = mybir.AxisListType

    nc = bacc.Bacc("TRN2", target_bir_lowering=False, debug=False, num_devices=NCORES)

    QSTEPS = (nsteps + NQ - 1) // NQ
    dWf_d = [
        nc.dram_tensor(f"dWf{q}", [128, QSTEPS * 48], f32, kind="ExternalInput").ap()
        for q in range(NQ)
    ]
    dZf_d = [
        nc.dram_tensor(f"dZf{q}", [128, QSTEPS * 48], f32, kind="ExternalInput").ap()
        for q in range(NQ)
    ]
    L1bg_d = nc.dram_tensor("L1bg", [GCH, GCH * 128], f32, kind="ExternalInput").ap()
    W1c_d = nc.dram_tensor("W1c", [2, 128], f32, kind="ExternalInput").ap()
    W2bd_d = nc.dram_tensor("W2bd", [128, 128], f32, kind="ExternalInput").ap()
    W3c_d = nc.dram_tensor("W3c", [128, 4], f32, kind="ExternalInput").ap()
    b1c_d = nc.dram_tensor("b1c", [128, 1], f32, kind="ExternalInput").ap()
    b2c_d = nc.dram_tensor("b2c", [128, 1], f32, kind="ExternalInput").ap()
    b3c_d = nc.dram_tensor("b3c", [1, 4], f32, kind="ExternalInput").ap()
    tvals_d = nc.dram_tensor("tvals", [1, nsteps], f32, kind="ExternalInput").ap()
    ones_col_d = nc.dram_tensor("ones_col", [128, 1], f32, kind="ExternalInput").ap()
    I128_d = nc.dram_tensor("I128", [128, 128], f32, kind="ExternalInput").ap()
    y_init_d = nc.dram_tensor("y_init", [16, 128], f32, kind="ExternalInput").ap()
    Y_init_d = nc.dram_tensor("Y_init", [128, 16], f32, kind="ExternalInput").ap()

    loss_out = nc.dram_tensor("loss_out", [1, 1], f32, kind="ExternalOutput").ap()
    if debug:
        y_out = nc.dram_tensor("y_out", [16, 128], f32, kind="ExternalOutput").ap()
        Y_out = nc.dram_tensor("Y_out", [128, 16], f32, kind="ExternalOutput").ap()

    SC_F = float((0.5 / DT) ** 0.5)  # fdt = (SC_F * qdt)^2 = 0.5*dt*q^2

    with tile.TileContext(nc) as tc:
        from contextlib import ExitStack

        with ExitStack() as ctx:
            cpool = ctx.enter_context(tc.tile_pool(name="const", bufs=1))
            h1pool = ctx.enter_context(tc.tile_pool(name="h1sb", bufs=3))
            h2pool = ctx.enter_context(tc.tile_pool(name="h2sb", bufs=3))
            epool = ctx.enter_context(tc.tile_pool(name="epil", bufs=3))
            pmm = ctx.enter_context(tc.tile_pool(name="pmm", bufs=2, space="PSUM"))
            pzq = ctx.enter_context(tc.tile_pool(name="pzq", bufs=2, space="PSUM"))
            ptr = ctx.enter_context(tc.tile_pool(name="ptr", bufs=1, space="PSUM"))
            ploss = ctx.enter_context(tc.tile_pool(name="ploss", bufs=1, space="PSUM"))

            # ------------- persistent SBUF tiles -------------
            dWs = [cpool.tile([128, QSTEPS * 48], f32, tag=f"dw{q}", name=f"dws{q}") for q in range(NQ)]
            dds = [cpool.tile([128, QSTEPS * 48], f32, tag=f"dz{q}", name=f"dds{q}") for q in range(NQ)]
            swpad = cpool.tile([128, nsteps * NCH * M5], f32, tag="swpad")
            W2bd_bf = cpool.tile([128, 128], bf16, tag="w2bd")
            L1bg_bf = cpool.tile([GCH, GCH * 128], bf16, tag="l1bg")
            W3_bf = cpool.tile([128, M5], bf16, tag="w3")
            W3_f = cpool.tile([128, 4], f32, tag="w3f")
            b1tab = cpool.tile([128, nsteps], f32, tag="b1tab")
            b1c_sb = cpool.tile([128, 1], f32, tag="b1c")
            b2c_sb = cpool.tile([128, 1], f32, tag="b2c")
            b3s = cpool.tile([1, M5], f32, tag="b3s")
            b3f = cpool.tile([1, 4], f32, tag="b3f")
            b3rep = cpool.tile([1, GCH * M5], bf16, tag="b3rep")
            ones_bf = cpool.tile([1, 128], bf16, tag="ones_bf")
            ones_col = cpool.tile([128, 1], f32, tag="ones_col")
            I128 = cpool.tile([128, 128], f32, tag="i128")
            I128bf = cpool.tile([128, 128], bf16, tag="i128bf")
            W1c_sb = cpool.tile([2, 128], f32, tag="w1c")
            tvals = cpool.tile([1, nsteps], f32, tag="tvals")
            yg_bf = [cpool.tile([GCH, 128], bf16, tag=f"ybf{g}", name=f"ygbf{g}") for g in range(NG)]
            Yacc = cpool.tile([128, 16], f32, tag="Yacc")
            ones_colbf = cpool.tile([128, 1], bf16, tag="ones_colbf")
            loss_sb = cpool.tile([1, 16], f32, tag="loss_sb")
            ysq = [cpool.tile([GCH, 128], f32, tag=f"ysq{g}", name=f"ysq{g}") for g in range(NG)]
            ee = [cpool.tile([128, GCH], f32, tag=f"ee{g}", name=f"ee{g}") for g in range(NG)]
            loss1 = cpool.tile([1, 1], f32, tag="loss1")

            loss_ps = ploss.tile([1, 16], f32, tag="loss")

            # ------------- init: DMAs -------------
            for q in range(NQ):
                nc.sync.dma_start(dWs[q][:], dWf_d[q][:])
                nc.sync.dma_start(dds[q][:], dZf_d[q][:])
            nc.gpsimd.dma_start(W2bd_bf[:], W2bd_d[:])
            nc.gpsimd.dma_start(L1bg_bf[:], L1bg_d[:])
            nc.sync.dma_start(W3_f[:], W3c_d[:])
            nc.sync.dma_start(b1c_sb[:], b1c_d[:])
            nc.sync.dma_start(b2c_sb[:], b2c_d[:])
            nc.sync.dma_start(b3f[:], b3c_d[:])
            nc.sync.dma_start(ones_col[:], ones_col_d[:])
            nc.sync.dma_start(I128[:], I128_d[:])
            nc.gpsimd.dma_start(I128bf[:], I128_d[:])
            nc.sync.dma_start(W1c_sb[:], W1c_d[:])
            nc.sync.dma_start(tvals[:], tvals_d[:])
            for g in range(NG):
                nc.gpsimd.dma_start(yg_bf[g][:], y_init_d[g * GCH : (g + 1) * GCH, :])
            nc.sync.dma_start(Yacc[:], Y_init_d[:, :])

            nc.vector.memset(ones_bf[:], 1.0)
            nc.vector.memset(ones_colbf[:], 1.0)
            nc.gpsimd.memset(swpad[:], 0.0)

            # ------------- init: compute -------------
            # b1tab[:, i] = b1c + t_i * W1[0, :]
            ps0 = pmm.tile([128, 1024], f32, tag="mm")
            nc.tensor.matmul(
                ps0[:, 0:nsteps], W1c_sb[0:1, :], tvals[0:1, :], start=True, stop=True
            )
            nc.scalar.activation(
                b1tab[:], ps0[:, 0:nsteps], AF.Identity, bias=b1c_sb[:, 0:1]
            )

            # W3 scaling: z-cols*sqrt(dt), q-col*dt, qs-col*dt*SC_F (bf16)
            nc.vector.tensor_scalar_mul(W3_bf[:, 0:3], W3_f[:, 0:3], float(SQRT_DT))
            nc.vector.tensor_scalar_mul(W3_bf[:, 3:4], W3_f[:, 3:4], float(DT))
            nc.vector.tensor_scalar_mul(W3_bf[:, 4:5], W3_f[:, 3:4], float(DT * SC_F))
            # b3 scaled pattern then replicate x8 into bf16 row [1, 40]
            nc.vector.tensor_scalar_mul(b3s[0:1, 0:3], b3f[0:1, 0:3], float(SQRT_DT))
            nc.vector.tensor_scalar_mul(b3s[0:1, 3:4], b3f[0:1, 3:4], float(DT))
            nc.vector.tensor_scalar_mul(b3s[0:1, 4:5], b3f[0:1, 3:4], float(DT * SC_F))
            nc.vector.tensor_copy(b3rep[0:1, 0:M5], b3s[0:1, :])
            nc.vector.tensor_copy(b3rep[0:1, M5 : 2 * M5], b3rep[0:1, 0:M5])
            nc.vector.tensor_copy(b3rep[0:1, 2 * M5 : 4 * M5], b3rep[0:1, 0 : 2 * M5])
            nc.vector.tensor_copy(b3rep[0:1, 4 * M5 : 8 * M5], b3rep[0:1, 0 : 4 * M5])

            # per-quarter prepass: dd = dW - dZ ; swpad q-slots
            for q in range(NQ):
                nsq = max(0, min(nsteps, (q + 1) * QSTEPS) - q * QSTEPS)
                if nsq == 0:
                    continue
                eng = nc.vector if q % 2 == 0 else nc.gpsimd
                eng.tensor_tensor(
                    dds[q][:, 0 : nsq * 48],
                    dWs[q][:, 0 : nsq * 48],
                    dds[q][:, 0 : nsq * 48],
                    op=ALU.subtract,
                )
                lo = q * QSTEPS * NCH * M5
                dst = swpad[:, lo : lo + nsq * NCH * M5].rearrange(
                    "p (s c m) -> p s c m", m=M5, c=NCH
                )[:, :, :, 3:4]
                src = dWs[q][:, 0 : nsq * 48].rearrange("p (s j) -> p s j", j=3)
                nc.vector.tensor_reduce(dst, src, axis=AX.X, op=ALU.add)
            nc.vector.tensor_scalar_mul(
                swpad[:].rearrange("p (s m) -> p s m", m=M5)[:, :, 3:4],
                swpad[:].rearrange("p (s m) -> p s m", m=M5)[:, :, 3:4],
                float(SIGMA0 * SQRT_DT),
            )

            # ------------- time-step loop (anti-phase group rounds) ----
            GW = GCH * M5  # 40 cols per group in zq layout
            for i in range(nsteps):
                qi, ri = divmod(i, QSTEPS)
                for g in range(NG):
                    h1ps = pmm.tile([128, 1024], f32, tag="mm", name=f"h1ps{i}_{g}")
                    h1sb = h1pool.tile([128, 1024], bf16, tag="h1", name=f"h1sb{i}_{g}")
                    h2ps = pmm.tile([128, 1024], f32, tag="mm", name=f"h2ps{i}_{g}")
                    h2sb = h2pool.tile([128, 1024], bf16, tag="h2", name=f"h2sb{i}_{g}")
                    zqf_ps = pzq.tile([128, GW], f32, tag="zq", name=f"zqps{i}_{g}")
                    zqf_sb = epool.tile([128, GW], bf16, tag=f"zqf{g}", name=f"zqf{i}_{g}")
                    zz = epool.tile([128, 48], f32, tag=f"zz{g}", name=f"zz{i}_{g}")
                    uurj = epool.tile([128, 16], f32, tag=f"uurj{g}", name=f"uurj{i}_{g}")
                    fdt = epool.tile([128, GCH], f32, tag=f"fdt{g}", name=f"fdt{i}_{g}")
                    uf = epool.tile([128, GCH], f32, tag=f"uf{g}", name=f"uf{i}_{g}")

                    # --- L1: h1[f, b] = W1r1[f]*y[b] (K=8 block-diag)
                    for c in range(GCH):
                        nc.tensor.matmul(
                            h1ps[:, c * 128 : (c + 1) * 128],
                            L1bg_bf[:, c * 128 : (c + 1) * 128],
                            yg_bf[g][:],
                            start=True,
                            stop=True,
                        )
                    # --- relu1 (+ per-step bias) -> bf16
                    nc.scalar.activation(h1sb[:], h1ps[:], AF.Relu, bias=b1tab[:, i : i + 1])

                    # --- L2
                    for k2 in range(2):
                        nc.tensor.matmul(
                            h2ps[:, k2 * 512 : (k2 + 1) * 512],
                            W2bd_bf[:],
                            h1sb[:, k2 * 512 : (k2 + 1) * 512],
                            start=True,
                            stop=True,
                        )
                    # --- relu2 -> bf16
                    if g == 0:
                        nc.vector.tensor_scalar(
                            h2sb[:], h2ps[:], b2c_sb[:, 0:1], 0.0,
                            op0=ALU.add, op1=ALU.max,
                        )
                    else:
                        nc.scalar.activation(h2sb[:], h2ps[:], AF.Relu, bias=b2c_sb[:, 0:1])

                    # --- L3: [z*sdt | q*dt | q*dt*SC_F] folded, + bias row
                    nc.tensor.matmul(
                        zqf_ps[:], ones_bf[0:1, :], b3rep[0:1, :],
                        start=True, stop=False, skip_group_check=True,
                    )
                    for c in range(GCH):
                        nc.tensor.matmul(
                            zqf_ps[:, c * M5 : (c + 1) * M5],
                            h2sb[:, c * 128 : (c + 1) * 128],
                            W3_bf[:],
                            start=False,
                            stop=(c == GCH - 1),
                            skip_group_check=True,
                        )

                    # --- epilogue: PSUM->SBUF add-copy folds in swp
                    swslice = swpad[:, (i * NCH + g * GCH) * M5 : (i * NCH + (g + 1) * GCH) * M5]
                    nc.vector.tensor_tensor(zqf_sb[:], zqf_ps[:], swslice, op=ALU.add)
                    zq5 = zqf_sb[:].rearrange("p (c m) -> p c m", m=M5)

                    # y update: incr = q*dt + swp (= zq5 col 3); y_bf += incr^T
                    incr = epool.tile([128, GCH], bf16, tag=f"incr{g}", name=f"incr{i}_{g}")
                    nc.vector.tensor_copy(
                        incr[:].rearrange("p (c o) -> p c o", o=1), zq5[:, :, 3:4])
                    incr_t = ptr.tile([GCH, 128], bf16, tag="tr", name=f"tr{i}_{g}")
                    nc.tensor.transpose(incr_t[:], incr[:], I128bf[:])
                    nc.vector.tensor_tensor(yg_bf[g][:], yg_bf[g][:], incr_t[:], op=ALU.add)

                    # slack: residual & Y accumulation
                    base = ri * 48 + g * 24
                    nc.gpsimd.tensor_tensor(
                        zz[:, 0:24].rearrange("p (c j) -> p c j", j=3),
                        zq5[:, :, 0:3],
                        dWs[qi][:, base : base + 24].rearrange("p (c j) -> p c j", j=3),
                        op=ALU.mult)
                    nc.gpsimd.tensor_tensor(
                        zz[:, 24:48].rearrange("p (c j) -> p c j", j=3),
                        zq5[:, :, 0:3],
                        dds[qi][:, base : base + 24].rearrange("p (c j) -> p c j", j=3),
                        op=ALU.mult)
                    nc.vector.tensor_reduce(
                        uurj[:].rearrange("p (h o) -> p h o", o=1),
                        zz[:].rearrange("p (h j) -> p h j", j=3),
                        axis=AX.X, op=ALU.add,
                    )
                    # loss: rr = rj^2 (bf16) ; loss_ps[g cols] += sum_p rr
                    rrb = epool.tile([128, GCH], bf16, tag=f"rrb{g}", name=f"rrb{i}_{g}")
                    nc.gpsimd.tensor_tensor(rrb[:], uurj[:, 8:16], uurj[:, 8:16], op=ALU.mult)
                    nc.tensor.matmul(
                        loss_ps[0:1, g * GCH : (g + 1) * GCH], ones_colbf[:], rrb[:],
                        start=(i == 0), stop=False, skip_group_check=True,
                    )
                    # Yacc += u - fdt
                    nc.gpsimd.tensor_tensor(
                        fdt[:].rearrange("p (c o) -> p c o", o=1),
                        zq5[:, :, 4:5], zq5[:, :, 4:5], op=ALU.mult)
                    nc.gpsimd.tensor_tensor(uf[:], uurj[:, 0:8], fdt[:], op=ALU.subtract)
                    nc.gpsimd.tensor_tensor(
                        Yacc[:, g * GCH : (g + 1) * GCH],
                        Yacc[:, g * GCH : (g + 1) * GCH],
                        uf[:], op=ALU.add)

            # ------------- terminal loss -------------
            for g in range(NG):
                nc.scalar.activation(ysq[g][:], yg_bf[g][:], AF.Square)
                ysq_ps = pzq.tile([128, GW], f32, tag="zq", name=f"term{g}")
                nc.tensor.matmul(
                    ysq_ps[:, 0:GCH], ysq[g][:], I128[0:GCH, 0:GCH], is_transpose=True
                )
                nc.vector.tensor_tensor(
                    ee[g][:], Yacc[:, g * GCH : (g + 1) * GCH], ysq_ps[:, 0:GCH],
                    op=ALU.subtract)
                eeb = epool.tile([128, GCH], bf16, tag=f"rrb{g}", name=f"eeb{g}")
                nc.scalar.activation(eeb[:], ee[g][:], AF.Square)
                nc.tensor.matmul(
                    loss_ps[0:1, g * GCH : (g + 1) * GCH], ones_colbf[:], eeb[:],
                    start=False, stop=(g == NG - 1), skip_group_check=True,
                )
            nc.vector.tensor_copy(loss_sb[:], loss_ps[:])
            nc.vector.tensor_reduce(
                loss1[:],
                loss_sb[0:1, :].rearrange("p (o c) -> p o c", o=1),
                axis=AX.X, op=ALU.add,
            )
            nc.vector.tensor_scalar_mul(loss1[:], loss1[:], 1.0 / B)
            nc.sync.dma_start(loss_out[:], loss1[:])
            if debug:
                for g in range(NG):
                    nc.sync.dma_start(y_out[g * GCH : (g + 1) * GCH, :], yg_bf[g][:])
                    nc.sync.dma_start(Y_out[:, g * GCH : (g + 1) * GCH], Yacc[:, g * GCH : (g + 1) * GCH])

    nc.compile()
    return nc


def _host_inputs(nsteps, y0, Y0, zW1, zb1, zW2, zb2, zW3, zb3, qW1, qb1, qW2, qb2, qW3, qb3, dW, dZ):
    """Per-core input maps. Layout/slicing only — no arithmetic on inputs."""
    f = np.float32
    QSTEPS = (nsteps + NQ - 1) // NQ
    W1row1 = np.concatenate([zW1[1], qW1[1]]).astype(f)
    L1bg = np.zeros((GCH, GCH * 128), f)
    for c in range(GCH):
        L1bg[c, c * 128 : (c + 1) * 128] = W1row1
    W1c = np.concatenate([zW1, qW1], axis=1).astype(f)  # (2,128)
    W2bd = np.zeros((128, 128), f)
    W2bd[0:64, 0:64] = zW2
    W2bd[64:128, 64:128] = qW2
    W3c = np.zeros((128, 4), f)
    W3c[0:64, 0:3] = zW3
    W3c[64:128, 3] = qW3[:, 0]
    b1c = np.concatenate([zb1, qb1]).astype(f).reshape(128, 1)
    b2c = np.concatenate([zb2, qb2]).astype(f).reshape(128, 1)
    b3c = np.concatenate([zb3, qb3]).astype(f).reshape(1, 4)
    tvals = (np.arange(nsteps) * DT).astype(f).reshape(1, nsteps)
    ones_col = np.ones((128, 1), f)
    I128 = np.eye(128, dtype=f)
    y_init = np.broadcast_to(np.asarray(y0, f).reshape(1, 1), (16, 128)).copy()
    Y_init = np.broadcast_to(np.asarray(Y0, f).reshape(1, 1), (128, 16)).copy()

    shared = dict(
        L1bg=L1bg, W1c=W1c, W2bd=W2bd, W3c=W3c, b1c=b1c, b2c=b2c, b3c=b3c,
        tvals=tvals, ones_col=ones_col, I128=I128,
        y_init=y_init, Y_init=Y_init,
    )

    in_maps = []
    for core in range(NCORES):
        o = core * BC
        m = dict(shared)
        for name, arr in (("dWf", dW), ("dZf", dZ)):
            # fold: [nsteps, 2048, 3] -> [128, nsteps*48],
            # col = i*48 + c*3 + j, path = c*128 + p
            x = np.ascontiguousarray(arr[:nsteps, o : o + BC, :]).astype(f)
            x = x.reshape(nsteps, NCH, 128, 3).transpose(2, 0, 1, 3)
            x = np.ascontiguousarray(x).reshape(128, nsteps * 48)
            for q in range(NQ):
                sl = x[:, q * QSTEPS * 48 : (q + 1) * QSTEPS * 48]
                buf = np.zeros((128, QSTEPS * 48), f)
                buf[:, : sl.shape[1]] = sl
                m[f"{name}{q}"] = buf
        in_maps.append(m)
    return in_maps


def _run(nsteps, inputs, debug=False):
    global LAST_EXEC_NS, LAST_RESULTS
    from concourse import bass_utils

    key = (nsteps, debug)
    if key not in _CACHE:
        _CACHE[key] = _build(nsteps, debug=debug)
    nc = _CACHE[key]

    in_maps = _host_inputs(nsteps, **inputs)
    trace = bool(os.environ.get("BASS_TRACE"))
    kwargs = {}
    if trace:
        import tempfile

        kwargs = dict(trace=True, tmpdir=tempfile.mkdtemp(prefix="bsde_trace_"))
    res = bass_utils.run_bass_kernel_spmd(
        nc, in_maps, core_ids=list(range(NCORES)), **kwargs
    )
    LAST_RESULTS = res
    LAST_EXEC_NS = res.exec_time_ns
    return res


def kernel(**inputs):
    inputs = {k: np.asarray(v, np.float32) for k, v in inputs.items()}
    res = _run(NSTEPS, inputs, debug=False)
    total = np.float32(0.0)
    for core in range(NCORES):
        total += res.results[core]["loss_out"][0, 0]
    return np.array(total, dtype=np.float32)
